# revision 31
# baseline (speedup 1.0000x reference)
"""Trainium2 Bass kernel for FlattenIntraCycleMoELayer (top-2 MoE + general path).

v2 strategy (see git/transcript for the trace analysis that motivated it):
  - Data-parallel over B (8 batteries per core).
  - gen_W is folded into every expert on the host (gates sum to 1), so each
    battery needs exactly one fused matmul pass: out = x @ (g1*We1' + g2*We2').
  - All DRAM layouts are partition-major so every bulk DMA is one
    contiguous-per-partition transfer (128 descriptors, full-rate HWDGE).
    Two HWDGE queues (SP=sync, ACT=scalar) stream: gating inputs first,
    then expert k-chunks, then per-battery x tiles.
  - Gating layer-1 runs in f32r (full fp32 operands, ~1 cyc/row at N=256),
    layer-2 in fp32; native Gelu activation.
  - Partial logits ([64,8] per core, d_ff-sharded) are exchanged with 7
    XOR-routed remote_dma_broadcast rounds (slot r of core j receives core
    j^r's partial; a sum is order-invariant) instead of a collective
    AllReduce; gated on the framework's bir-kernel barrier.
  - Top-2 select/softmax on a [8,E] tile, gates broadcast to 128 partitions
    via a K=1 matmul (ones ⊗ flat-gates), per-k-tile combine on
    scalar+vector chases the EXP DMA stream, fused matmuls run kt-outer so
    the first matmul needs only combine-chunk 0.
  - Output is evicted in bf16 and upcast on the host.

Host-side prep only reshapes/pads/casts/adds weight tensors (no forward-pass
math on host).
"""

import os
import numpy as np
import ml_dtypes


def _ensure_import_path():
    try:
        import concourse  # noqa: F401
    except ImportError:
        import sys
        for p in ("/opt/trn_rl_repo", "/root/.axon_site/_ro/trn_rl_repo"):
            if p not in sys.path:
                sys.path.insert(0, p)
        import concourse  # noqa: F401


_ensure_import_path()

import concourse.bass as bass  # noqa: E402
import concourse.tile as tile  # noqa: E402
from concourse import mybir  # noqa: E402
from concourse.bass import ds, ts  # noqa: E402
from concourse.alu_op_type import AluOpType  # noqa: E402
from concourse.masks import make_identity  # noqa: E402

BF16 = mybir.dt.bfloat16
F32 = mybir.dt.float32
U32 = mybir.dt.uint32
F32R = mybir.dt.float32r

# Problem shape constants (hardcoded per contest rules).
B, L, C, F = 64, 512, 3, 300
CF = C * F              # 900
KP = 1024               # padded contraction dim (900 data + 1 ones + 123 zero)
KT = KP // 128          # 8 k-tiles
D = 512                 # d_model
E = 8                   # experts
NCORES = 8
BPC = B // NCORES       # 8 batteries per core
DLLM = 4096
GK = 4224               # padded gating contraction = 33*128
GKT = GK // 128         # 33
DFF = 2048
DFFC = DFF // NCORES    # 256 per-core d_ff chunk
EPS = 1e-9
MT = L // 128           # 4 m-tiles per battery

USE_CC = bool(os.environ.get("MOE_CC"))   # fallback: collective AllReduce

_POSTHOC = {}   # instruction handles for waits attached after Tile scheduling


def build_program(nc):
    from contextlib import ExitStack

    xt = nc.dram_tensor("xt", [BPC, 128, KT, L], BF16, kind="ExternalInput")
    ew = nc.dram_tensor("ew", [128, KT, E, D], BF16, kind="ExternalInput")
    gint = nc.dram_tensor("gint", [128, GKT, B], F32R, kind="ExternalInput")
    w1c = nc.dram_tensor("w1c", [128, GKT, DFFC], F32R, kind="ExternalInput")
    w2c = nc.dram_tensor("w2c", [128, DFFC // 128, E], F32, kind="ExternalInput")
    selt = nc.dram_tensor("selt", [B, BPC], F32, kind="ExternalInput")
    b2 = nc.dram_tensor("b2", [1, E], F32, kind="ExternalInput")
    out = nc.dram_tensor("out", [BPC, MT, 128, D], BF16, kind="ExternalOutput")

    xt_ap = xt.ap()
    ew_ap = ew.ap()
    out_ap = out.ap()

    with tile.TileContext(nc) as tc, ExitStack() as ctx:
        singles = ctx.enter_context(tc.tile_pool(name="singles", bufs=1))
        gpool = ctx.enter_context(tc.tile_pool(name="gate", bufs=1))
        wbpool = ctx.enter_context(tc.tile_pool(name="wbs", bufs=2))
        scpool = ctx.enter_context(tc.tile_pool(name="scratch", bufs=2))
        opool = ctx.enter_context(tc.tile_pool(name="outs", bufs=4))
        gps_ctx = ExitStack()
        gps = gps_ctx.enter_context(tc.tile_pool(name="gpsum", bufs=1, space="PSUM"))
        w1_ctx = ExitStack()
        w1pool = w1_ctx.enter_context(tc.tile_pool(name="w1s", bufs=1))

        # ------------- DMA queue assignment ---------------------------------
        # scalar (ACT hwdge): ONLY two early W1 chunks — its ring waits must
        #   not head-of-line-block the gating/combine compute it runs later.
        # sync (SP hwdge): gint, the other W1 chunks, EXP k-chunks, xb0,
        #   then all output evictions.
        # gpsimd (sw dge): small tensors, the logits exchange, xb1..7.
        w1_sb = w1pool.tile([128, GKT, DFFC], F32R)
        w1ap = w1c.ap()
        nc.scalar.dma_start(out=w1_sb[:, 0:9, :], in_=w1ap[:, 0:9, :])
        nc.scalar.dma_start(out=w1_sb[:, 17:25, :], in_=w1ap[:, 17:25, :])

        ginT_sb = w1pool.tile([128, GKT, B], F32R)
        nc.sync.dma_start(out=ginT_sb, in_=gint.ap())
        nc.sync.dma_start(out=w1_sb[:, 9:17, :], in_=w1ap[:, 9:17, :])
        nc.sync.dma_start(out=w1_sb[:, 25:33, :], in_=w1ap[:, 25:33, :])

        EXP_sb = singles.tile([128, KT, E, D], BF16)
        xt_sb = singles.tile([128, BPC, KT, L], BF16)
        nc.sync.dma_start(out=EXP_sb[:, 0, :, :], in_=ew_ap[:, 0, :, :])
        nc.sync.dma_start(out=xt_sb[:, 0, :, :], in_=xt_ap[0])
        for kt in range(1, KT):
            nc.sync.dma_start(out=EXP_sb[:, kt, :, :], in_=ew_ap[:, kt, :, :])

        # small gating tensors on gpsimd (software DGE queue)
        w2_sb = gpool.tile([128, DFFC // 128, E], F32)
        nc.gpsimd.dma_start(out=w2_sb, in_=w2c.ap())
        selt_sb = gpool.tile([B, BPC], F32)
        nc.gpsimd.dma_start(out=selt_sb, in_=selt.ap())
        b2_ap = b2.ap()
        b2bc = gpool.tile([B, E], F32)
        nc.gpsimd.dma_start(
            out=b2bc,
            in_=bass.AP(tensor=b2_ap.tensor, offset=b2_ap.offset,
                        ap=[[0, B]] + list(b2_ap.ap[1:])),
        )
        ident = singles.tile([128, 128], F32)
        make_identity(nc, ident)
        onesrow = singles.tile([1, 128], F32)
        nc.gpsimd.memset(onesrow, 1.0)

        # exchange buffers; partial's pad partitions are zeroed (they are
        # broadcast to peers).  gath is never memset — remote writes from
        # peers land in it and a local memset could race them.
        partial = gpool.tile([128, E], F32)
        nc.gpsimd.memset(partial, 0.0)
        gath = gpool.tile([128, NCORES, E], F32)

        # ------------- gating layer 1 (f32r), chasing the W1 stream -------
        psum_h = gps.tile([B, DFFC], F32, bufs=1)
        for kt in range(GKT):
            nc.tensor.matmul(
                out=psum_h,
                lhsT=ginT_sb[:, kt, :],
                rhs=w1_sb[:, kt, :],
                start=(kt == 0), stop=(kt == GKT - 1),
            )
        w1_ctx.close()  # release W1 SBUF
        # gelu, tanh approx (matches jax.nn.gelu default):
        #   h = 0.5*x*(1 + tanh(0.79788456*(x + 0.044715*x^3)))
        g_x = gpool.tile([B, DFFC], F32)
        nc.vector.tensor_copy(out=g_x, in_=psum_h)
        g_x2 = gpool.tile([B, DFFC], F32)
        nc.vector.tensor_tensor(out=g_x2, in0=g_x, in1=g_x, op=AluOpType.mult)
        g_p = gpool.tile([B, DFFC], F32)
        nc.vector.tensor_scalar(g_p, g_x2, 0.044715, 1.0,
                                AluOpType.mult, AluOpType.add)
        g_u = gpool.tile([B, DFFC], F32)
        nc.vector.tensor_tensor(out=g_u, in0=g_x, in1=g_p, op=AluOpType.mult)
        g_t = gpool.tile([B, DFFC], F32)
        nc.scalar.activation(out=g_t, in_=g_u,
                             func=mybir.ActivationFunctionType.Tanh,
                             scale=0.7978845608028654)
        g_q = gpool.tile([B, DFFC], F32)
        nc.vector.tensor_scalar(g_q, g_t, 1.0, 0.5,
                                AluOpType.add, AluOpType.mult)
        h_sb = gpool.tile([B, DFFC], F32)
        nc.vector.tensor_tensor(out=h_sb, in0=g_x, in1=g_q, op=AluOpType.mult)
        # transpose h -> hT [128, 2, B]
        hT_sb = gpool.tile([128, DFFC // 128, B], F32)
        for j in range(DFFC // 128):
            pst = gps.tile([128, B], F32, bufs=2, tag="pst")
            nc.tensor.transpose(
                out=pst, in_=h_sb[:, j * 128:(j + 1) * 128], identity=ident[:B, :B]
            )
            nc.vector.tensor_copy(out=hT_sb[:, j, :], in_=pst)
        # layer 2 partial logits [B, E]
        psum_l = gps.tile([B, E], F32, bufs=2, tag="pst")
        for j in range(DFFC // 128):
            nc.tensor.matmul(
                out=psum_l, lhsT=hT_sb[:, j, :], rhs=w2_sb[:, j, :],
                start=(j == 0), stop=(j == DFFC // 128 - 1),
            )
        nc.vector.tensor_copy(out=partial[:B, :], in_=psum_l)

        # ------------- logits exchange: 7 XOR-routed remote bcasts --------
        s1 = gpool.tile([B, E], F32)
        if USE_CC:
            dpool = ctx.enter_context(tc.tile_pool(name="dram", bufs=1, space="DRAM"))
            ar_in = dpool.tile([B, E], F32)
            nc.gpsimd.dma_start(out=ar_in, in_=partial[:B, :])
            ar_out = dpool.tile([B, E], F32, addr_space="Shared")
            nc.gpsimd.collective_compute(
                "AllReduce", AluOpType.add,
                replica_groups=[list(range(NCORES))],
                ins=[ar_in], outs=[ar_out],
            )
            nc.gpsimd.dma_start(out=s1, in_=ar_out)
        else:
            # self contribution into slot 0 (receiver j's slot r <- core j^r)
            nc.vector.tensor_copy(out=gath[:, 0, :], in_=partial)
            rsem = nc.alloc_semaphore("logit_rsem")
            lsem = nc.alloc_semaphore("logit_lsem")
            for r in range(1, NCORES):
                rd = [None] * 8
                rd[r] = (0, r)
                nc.gpsimd.remote_dma_broadcast(
                    out_ap=gath[:, r, :], in_ap=partial,
                    remote_sem=rsem, local_sem=lsem, rdests=rd,
                )
            trig = nc.gpsimd.trigger_dma(count=None)
            # sum slots: s1[b] = sum_r gath[b, r, :]
            s4 = gpool.tile([B, 4, E], F32)
            sum_inst = nc.vector.tensor_tensor(
                out=s4, in0=gath[:B, 0:4, :], in1=gath[:B, 4:8, :],
                op=AluOpType.add)
            s2 = gpool.tile([B, 2, E], F32)
            nc.vector.tensor_tensor(out=s2, in0=s4[:, 0:2, :],
                                    in1=s4[:, 2:4, :], op=AluOpType.add)
            nc.vector.tensor_tensor(out=s1, in0=s2[:, 0, :], in1=s2[:, 1, :],
                                    op=AluOpType.add)
            # The barrier wait (on trig) and the remote-arrival wait (on the
            # first gath reader) are satisfied by OTHER cores, which the
            # Tile scheduler's single-core sim cannot model — attached after
            # scheduling in make_nc().
            _POSTHOC["trig0"] = trig
            _POSTHOC["sum_waits"] = [(sum_inst, rsem, 2 * (NCORES - 1))]

        # xb1..7 stream on the gpsimd software queue, issued after the
        # exchange descriptors so they cannot delay it.
        for b in range(1, BPC):
            nc.gpsimd.dma_start(out=xt_sb[:, b, :, :], in_=xt_ap[b])

        # ------------- select my batteries, top-2 gates -------------------
        logits_all = gpool.tile([B, E], F32)
        nc.vector.tensor_tensor(out=logits_all, in0=s1, in1=b2bc,
                                op=AluOpType.add)
        # project my 8 batteries to partitions 0..7
        psum_sel = gps.tile([BPC, E], F32, bufs=2, tag="pst")
        nc.tensor.matmul(out=psum_sel, lhsT=selt_sb, rhs=logits_all,
                         start=True, stop=True)
        logits_my = gpool.tile([BPC, E], F32)
        nc.vector.tensor_copy(out=logits_my, in_=psum_sel)

        sorted8 = gpool.tile([BPC, E], F32)
        sidx = gpool.tile([BPC, E], U32)
        nc.vector.max(out=sorted8, in_=logits_my)
        nc.vector.max_index(out=sidx, in_max=sorted8, in_values=logits_my)
        negmax = gpool.tile([BPC, 1], F32)
        nc.vector.tensor_scalar_mul(negmax, sorted8[:, 0:1], -1.0)
        q = gpool.tile([BPC, E], F32)
        nc.scalar.activation(out=q, in_=sorted8,
                             func=mybir.ActivationFunctionType.Exp,
                             bias=negmax, scale=1.0)
        zsum = gpool.tile([BPC, 1], F32)
        nc.vector.reduce_sum(out=zsum, in_=q, axis=mybir.AxisListType.X)
        t12 = gpool.tile([BPC, 1], F32)
        nc.vector.tensor_tensor(out=t12, in0=q[:, 0:1], in1=q[:, 1:2],
                                op=AluOpType.add)
        den = gpool.tile([BPC, 1], F32)
        nc.vector.scalar_tensor_tensor(out=den, in0=zsum, scalar=EPS, in1=t12,
                                       op0=AluOpType.mult, op1=AluOpType.add)
        rden = gpool.tile([BPC, 1], F32)
        nc.vector.reciprocal(out=rden, in_=den)
        g12 = gpool.tile([BPC, 2], F32)
        nc.vector.tensor_scalar_mul(g12, q[:, 0:2], rden)

        # broadcast gates to all partitions: [8,2] -fold-> [1,16] -mm-> [128,16]
        gflat = gpool.tile([1, 2 * BPC], F32)
        nc.gpsimd.dma_start(out=gflat, in_=g12)
        psum_bc = gps.tile([128, 2 * BPC], F32, bufs=2, tag="pst")
        nc.tensor.matmul(out=psum_bc, lhsT=onesrow, rhs=gflat,
                         start=True, stop=True)
        bcastG = gpool.tile([128, 2 * BPC], F32)
        nc.scalar.activation(out=bcastG, in_=psum_bc,
                             func=mybir.ActivationFunctionType.Copy)
        bcastGbf = gpool.tile([128, 2 * BPC], BF16)
        nc.vector.tensor_copy(out=bcastGbf, in_=psum_bc)
        gps_ctx.close()  # release gating PSUM banks

        mps = ctx.enter_context(tc.tile_pool(name="mpsum", bufs=8, space="PSUM"))

        # ------------- per-battery combine + fused matmuls ----------------
        def _vload(eng, ap, name):
            reg = eng.alloc_register(name)
            eng.reg_load(reg, ap)
            val = eng.snap(reg, donate=True)
            return nc.s_assert_within(val, 0, E - 1, skip_runtime_assert=True)

        wbs = {}

        def combine(b):
            """wb[kt] = g1*EXP[kt, e1] + g2*EXP[kt, e2], chunked per k-tile."""
            rv1 = _vload(nc.scalar, sidx[b:b + 1, 0:1], f"e1_{b}")
            rv2 = _vload(nc.vector, sidx[b:b + 1, 1:2], f"e2_{b}")
            wb = wbpool.tile([128, KT, D], BF16)
            for kt in range(KT):
                tsc = scpool.tile([128, D], BF16, tag="tsc", bufs=4)
                src1 = EXP_sb[:, kt, ds(rv1, 1), :].rearrange("p o d -> p (o d)")
                nc.scalar.activation(
                    out=tsc, in_=src1,
                    func=mybir.ActivationFunctionType.Copy,
                    scale=bcastG[:, 2 * b:2 * b + 1],
                )
                src2 = EXP_sb[:, kt, ds(rv2, 1), :].rearrange("p o d -> p (o d)")
                nc.vector.scalar_tensor_tensor(
                    out=wb[:, kt, :], in0=src2,
                    scalar=bcastGbf[:, 2 * b + 1:2 * b + 2],
                    in1=tsc, op0=AluOpType.mult, op1=AluOpType.add,
                )
            wbs[b] = wb

        def emit_battery(b):
            wb = wbs[b]
            psums = [mps.tile([128, D], F32, tag=f"pm{b % 2}_{m}", bufs=1,
                              name=f"pm_{b}_{m}")
                     for m in range(MT)]
            for kt in range(KT):
                for m in range(MT):
                    nc.tensor.matmul(
                        out=psums[m], lhsT=xt_sb[:, b, kt, ts(m, 128)],
                        rhs=wb[:, kt, :],
                        start=(kt == 0), stop=(kt == KT - 1),
                    )
            for m in range(MT):
                osb = opool.tile([128, D], BF16)
                nc.vector.tensor_copy(out=osb, in_=psums[m])
                nc.sync.dma_start(out=out_ap[b, m], in_=osb)

        combine(0)
        for b in range(BPC):
            if b + 1 < BPC:
                combine(b + 1)
            emit_battery(b)


def make_nc():
    from concourse import bacc
    nc = bacc.Bacc("TRN2", target_bir_lowering=False, debug=False,
                   num_devices=NCORES)
    build_program(nc)
    if not USE_CC:
        # Register the prelude AllGather (keeps NEFF collective init +
        # preamble barrier); nobody waits on its completion.  Attach each
        # ladder round's remote-arrival wait (2 increments per arrival) to
        # its local add; the Tile scheduler's single-core sim must not see
        # these waits (they cannot be satisfied locally).
        nc._bir_kernel_barrier_sem_replica_groups.append(set(range(NCORES)))
        # execution starts are skewed across the 8 cores (PJRT dispatch);
        # the first send must wait until every core has entered the kernel
        # (prelude AllGather completion) or its increments are lost.
        _POSTHOC["trig0"].wait_op(
            nc._bir_kernel_barrier_sem, nc.bir_kernel_barrier_sem_inc,
            "sem-ge", check=False)
        for add, sem, val in _POSTHOC["sum_waits"]:
            add.wait_op(sem, val, "sem-ge", check=False)
    nc.finalize()
    return nc


def prep_inputs(cycle_curve_data, cycle_numbers, DKP_embeddings,
                gate_W1, gate_b1, gate_W2, gate_b2,
                expert_W, expert_b, gen_W, gen_b):
    """Host-side layout prep. Returns per-core in_maps list."""
    f32 = np.float32
    bf16 = ml_dtypes.bfloat16
    x = np.asarray(cycle_curve_data, dtype=f32).reshape(B, L, CF)

    # xt[b, p, kt, l] = xpad[b, kt*128+p, l]
    xpad = np.zeros((B, KP, L), dtype=bf16)
    xpad[:, :CF, :] = x.transpose(0, 2, 1).astype(bf16)
    xpad[:, CF, :] = np.asarray(1.0, dtype=bf16)
    xt_all = np.ascontiguousarray(
        xpad.reshape(B, KT, 128, L).transpose(0, 2, 1, 3))

    # fold general path into every expert (gates sum to 1)
    ew_f = np.asarray(expert_W, dtype=f32) + np.asarray(gen_W, dtype=f32)[None]
    eb_f = np.asarray(expert_b, dtype=f32) + np.asarray(gen_b, dtype=f32)[None]
    ew_p = np.zeros((E, KP, D), dtype=bf16)
    ew_p[:, :CF, :] = ew_f.astype(bf16)
    ew_p[:, CF, :] = eb_f.astype(bf16)
    # ew_dram[p, kt, e, d] = ew_p[e, kt*128+p, d]
    ew_dram = np.ascontiguousarray(
        ew_p.reshape(E, KT, 128, D).transpose(2, 1, 0, 3))

    gint = np.zeros((GK, B), dtype=f32)
    gint[:DLLM, :] = np.asarray(DKP_embeddings, dtype=f32).T
    gint[DLLM, :] = np.asarray(cycle_numbers, dtype=f32)[:, 0]
    gint[DLLM + 1, :] = 1.0
    gint_dram = np.ascontiguousarray(
        gint.reshape(GKT, 128, B).transpose(1, 0, 2))

    w1p = np.zeros((GK, DFF), dtype=f32)
    w1p[:DLLM + 1, :] = np.asarray(gate_W1, dtype=f32)
    w1p[DLLM + 1, :] = np.asarray(gate_b1, dtype=f32)

    w2 = np.asarray(gate_W2, dtype=f32)
    b2v = np.asarray(gate_b2, dtype=f32).reshape(1, E)

    in_maps = []
    for c in range(NCORES):
        w1slice = w1p[:, c * DFFC:(c + 1) * DFFC]
        w1_dram = np.ascontiguousarray(
            w1slice.reshape(GKT, 128, DFFC).transpose(1, 0, 2))
        w2slice = w2[c * DFFC:(c + 1) * DFFC, :]
        w2_dram = np.ascontiguousarray(
            w2slice.reshape(DFFC // 128, 128, E).transpose(1, 0, 2))
        sel = np.zeros((B, BPC), dtype=f32)
        for i in range(BPC):
            sel[c * BPC + i, i] = 1.0
        in_maps.append({
            "xt": np.ascontiguousarray(xt_all[c * BPC:(c + 1) * BPC]),
            "ew": ew_dram,
            "gint": gint_dram,
            "w1c": w1_dram,
            "w2c": w2_dram,
            "selt": sel,
            "b2": b2v,
        })
    return in_maps


_CACHED = {}


def run(inputs, trace=False, tmpdir=None):
    """Run on the 8 NeuronCores; returns (full_output, BassKernelResults)."""
    from concourse import bass_utils
    in_maps = prep_inputs(**inputs)
    nc = _CACHED.get("nc")
    if nc is None:
        nc = make_nc()
        _CACHED["nc"] = nc
    res = bass_utils.run_bass_kernel_spmd(
        nc, in_maps, core_ids=list(range(NCORES)), trace=trace, tmpdir=tmpdir
    )
    outs = [np.asarray(r["out"]).reshape(BPC, L, D) for r in res.results]
    full = np.concatenate(outs, axis=0).astype(np.float32)
    return full, res


def kernel(**inputs):
    full, _ = run(inputs, trace=False)
    return full


# revision 35
# speedup vs baseline: 1.4054x; 1.4054x over previous
"""Trainium2 Bass kernel for FlattenIntraCycleMoELayer (top-2 MoE + general path).

v2 strategy (see git/transcript for the trace analysis that motivated it):
  - Data-parallel over B (8 batteries per core).
  - gen_W is folded into every expert on the host (gates sum to 1), so each
    battery needs exactly one fused matmul pass: out = x @ (g1*We1' + g2*We2').
  - All DRAM layouts are partition-major so every bulk DMA is one
    contiguous-per-partition transfer (128 descriptors, full-rate HWDGE).
    Two HWDGE queues (SP=sync, ACT=scalar) stream: gating inputs first,
    then expert k-chunks, then per-battery x tiles.
  - Gating layer-1 runs in f32r (full fp32 operands, ~1 cyc/row at N=256),
    layer-2 in fp32; native Gelu activation.
  - Partial logits ([64,8] per core, d_ff-sharded) are exchanged with 7
    XOR-routed remote_dma_broadcast rounds (slot r of core j receives core
    j^r's partial; a sum is order-invariant) instead of a collective
    AllReduce; gated on the framework's bir-kernel barrier.
  - Top-2 select/softmax on a [8,E] tile, gates broadcast to 128 partitions
    via a K=1 matmul (ones ⊗ flat-gates), per-k-tile combine on
    scalar+vector chases the EXP DMA stream, fused matmuls run kt-outer so
    the first matmul needs only combine-chunk 0.
  - Output is evicted in bf16 and upcast on the host.

Host-side prep only reshapes/pads/casts/adds weight tensors (no forward-pass
math on host).
"""

import os
import numpy as np
import ml_dtypes


def _ensure_import_path():
    try:
        import concourse  # noqa: F401
    except ImportError:
        import sys
        for p in ("/opt/trn_rl_repo", "/root/.axon_site/_ro/trn_rl_repo"):
            if p not in sys.path:
                sys.path.insert(0, p)
        import concourse  # noqa: F401


_ensure_import_path()

import concourse.bass as bass  # noqa: E402
import concourse.tile as tile  # noqa: E402
from concourse import mybir  # noqa: E402
from concourse.bass import ds, ts  # noqa: E402
from concourse.alu_op_type import AluOpType  # noqa: E402
from concourse.masks import make_identity  # noqa: E402
from concourse.tile import add_dep_helper  # noqa: E402

BF16 = mybir.dt.bfloat16
F32 = mybir.dt.float32
U32 = mybir.dt.uint32
F32R = mybir.dt.float32r

# Problem shape constants (hardcoded per contest rules).
B, L, C, F = 64, 512, 3, 300
CF = C * F              # 900
KP = 1024               # padded contraction dim (900 data + 1 ones + 123 zero)
KT = KP // 128          # 8 k-tiles
D = 512                 # d_model
E = 8                   # experts
NCORES = 8
BPC = B // NCORES       # 8 batteries per core
DLLM = 4096
GK = 4224               # padded gating contraction = 33*128
GKT = GK // 128         # 33
DFF = 2048
DFFC = DFF // NCORES    # 256 per-core d_ff chunk
EPS = 1e-9
MT = L // 128           # 4 m-tiles per battery

USE_CC = bool(os.environ.get("MOE_CC"))   # fallback: collective AllReduce

_POSTHOC = {}   # instruction handles for waits attached after Tile scheduling


def build_program(nc):
    from contextlib import ExitStack

    xt = nc.dram_tensor("xt", [BPC, 128, KT, L], BF16, kind="ExternalInput")
    ew = nc.dram_tensor("ew", [128, KT, E, D], BF16, kind="ExternalInput")
    gint = nc.dram_tensor("gint", [128, GKT, B], F32R, kind="ExternalInput")
    w1c = nc.dram_tensor("w1c", [128, GKT, DFFC], F32R, kind="ExternalInput")
    w2c = nc.dram_tensor("w2c", [128, DFFC // 128, E], F32, kind="ExternalInput")
    selt = nc.dram_tensor("selt", [B, BPC], F32, kind="ExternalInput")
    b2 = nc.dram_tensor("b2", [1, E], F32, kind="ExternalInput")
    out = nc.dram_tensor("out", [BPC, MT, 128, D], BF16, kind="ExternalOutput")

    xt_ap = xt.ap()
    ew_ap = ew.ap()
    out_ap = out.ap()

    with tile.TileContext(nc) as tc, ExitStack() as ctx:
        singles = ctx.enter_context(tc.tile_pool(name="singles", bufs=1))
        gpool = ctx.enter_context(tc.tile_pool(name="gate", bufs=1))
        wbpool = ctx.enter_context(tc.tile_pool(name="wbs", bufs=2))
        scpool = ctx.enter_context(tc.tile_pool(name="scratch", bufs=2))
        opool = ctx.enter_context(tc.tile_pool(name="outs", bufs=4))
        gps_ctx = ExitStack()
        gps = gps_ctx.enter_context(tc.tile_pool(name="gpsum", bufs=1, space="PSUM"))
        w1_ctx = ExitStack()
        w1pool = w1_ctx.enter_context(tc.tile_pool(name="w1s", bufs=1))

        # ------------- DMA queue assignment ---------------------------------
        # scalar (ACT hwdge): ONLY two early W1 chunks — its ring waits must
        #   not head-of-line-block the gating/combine compute it runs later.
        # sync (SP hwdge): gint, the other W1 chunks, EXP k-chunks, xb0,
        #   then all output evictions.
        # gpsimd (sw dge): small tensors, the logits exchange, xb1..7.
        # gint first at full bandwidth (L1's lhsT), then the four W1 chunks
        # in parallel across both queues, then EXP k-chunks serially, with
        # xb0 alongside, then xb1..7 — all explicitly chained: HWDGE ring
        # entries execute concurrently, so priority requires dependencies.
        ginT_sb = w1pool.tile([128, GKT, B], F32R)
        gin_dma = nc.sync.dma_start(out=ginT_sb, in_=gint.ap())

        w1_sb = w1pool.tile([128, GKT, DFFC], F32R)
        w1ap = w1c.ap()
        w1_dmas = []
        for i, (a, b_, eng) in enumerate([(0, 9, nc.scalar), (9, 17, nc.sync),
                                          (17, 25, nc.scalar),
                                          (25, 33, nc.sync)]):
            dma = eng.dma_start(out=w1_sb[:, a:b_, :], in_=w1ap[:, a:b_, :])
            add_dep_helper(dma.ins, gin_dma.ins, sync=False,
                           reason="W1 after gint")
            w1_dmas.append(dma)

        EXP_sb = singles.tile([128, KT, E, D], BF16)
        xt_sb = singles.tile([128, BPC, KT, L], BF16)
        prev = None
        for kt in range(KT):
            dma = nc.sync.dma_start(out=EXP_sb[:, kt, :, :],
                                    in_=ew_ap[:, kt, :, :])
            if prev is None:
                for w in w1_dmas:
                    add_dep_helper(dma.ins, w.ins, sync=False,
                                   reason="EXP after W1")
            else:
                add_dep_helper(dma.ins, prev.ins, sync=False,
                               reason="EXP chain")
            prev = dma
            if kt == 0:
                xb0 = nc.sync.dma_start(out=xt_sb[:, 0, :, :], in_=xt_ap[0])
                for w in w1_dmas:
                    add_dep_helper(xb0.ins, w.ins, sync=False,
                                   reason="xb0 after W1")
        for b in range(1, BPC):
            dma = nc.sync.dma_start(out=xt_sb[:, b, :, :], in_=xt_ap[b])
            add_dep_helper(dma.ins, prev.ins, sync=False, reason="xb chain")
            prev = dma

        # small gating tensors on gpsimd (software DGE queue)
        w2_sb = gpool.tile([128, DFFC // 128, E], F32)
        nc.gpsimd.dma_start(out=w2_sb, in_=w2c.ap())
        selt_sb = gpool.tile([B, BPC], F32)
        nc.gpsimd.dma_start(out=selt_sb, in_=selt.ap())
        b2_ap = b2.ap()
        b2bc = gpool.tile([B, E], F32)
        nc.gpsimd.dma_start(
            out=b2bc,
            in_=bass.AP(tensor=b2_ap.tensor, offset=b2_ap.offset,
                        ap=[[0, B]] + list(b2_ap.ap[1:])),
        )
        ident = singles.tile([128, 128], F32)
        make_identity(nc, ident)
        onesrow = singles.tile([1, 128], F32)
        nc.gpsimd.memset(onesrow, 1.0)

        # exchange buffers; partial's pad partitions are zeroed (they are
        # broadcast to peers).  gath is never memset — remote writes from
        # peers land in it and a local memset could race them.
        partial = gpool.tile([128, E], F32)
        nc.gpsimd.memset(partial, 0.0)
        gath = gpool.tile([128, NCORES, E], F32)

        # ------------- gating layer 1 (f32r), chasing the W1 stream -------
        psum_h = gps.tile([B, DFFC], F32, bufs=1)
        for kt in range(GKT):
            nc.tensor.matmul(
                out=psum_h,
                lhsT=ginT_sb[:, kt, :],
                rhs=w1_sb[:, kt, :],
                start=(kt == 0), stop=(kt == GKT - 1),
            )
        w1_ctx.close()  # release W1 SBUF
        # gelu, tanh approx (matches jax.nn.gelu default):
        #   h = 0.5*x*(1 + tanh(0.79788456*(x + 0.044715*x^3)))
        g_x = gpool.tile([B, DFFC], F32)
        nc.vector.tensor_copy(out=g_x, in_=psum_h)
        g_x2 = gpool.tile([B, DFFC], F32)
        nc.vector.tensor_tensor(out=g_x2, in0=g_x, in1=g_x, op=AluOpType.mult)
        g_p = gpool.tile([B, DFFC], F32)
        nc.vector.tensor_scalar(g_p, g_x2, 0.044715, 1.0,
                                AluOpType.mult, AluOpType.add)
        g_u = gpool.tile([B, DFFC], F32)
        nc.vector.tensor_tensor(out=g_u, in0=g_x, in1=g_p, op=AluOpType.mult)
        g_t = gpool.tile([B, DFFC], F32)
        nc.scalar.activation(out=g_t, in_=g_u,
                             func=mybir.ActivationFunctionType.Tanh,
                             scale=0.7978845608028654)
        g_q = gpool.tile([B, DFFC], F32)
        nc.vector.tensor_scalar(g_q, g_t, 1.0, 0.5,
                                AluOpType.add, AluOpType.mult)
        h_sb = gpool.tile([B, DFFC], F32)
        nc.vector.tensor_tensor(out=h_sb, in0=g_x, in1=g_q, op=AluOpType.mult)
        # transpose h -> hT [128, 2, B]
        hT_sb = gpool.tile([128, DFFC // 128, B], F32)
        for j in range(DFFC // 128):
            pst = gps.tile([128, B], F32, bufs=2, tag="pst")
            nc.tensor.transpose(
                out=pst, in_=h_sb[:, j * 128:(j + 1) * 128], identity=ident[:B, :B]
            )
            nc.vector.tensor_copy(out=hT_sb[:, j, :], in_=pst)
        # layer 2 partial logits [B, E]
        psum_l = gps.tile([B, E], F32, bufs=2, tag="pst")
        for j in range(DFFC // 128):
            nc.tensor.matmul(
                out=psum_l, lhsT=hT_sb[:, j, :], rhs=w2_sb[:, j, :],
                start=(j == 0), stop=(j == DFFC // 128 - 1),
            )
        nc.vector.tensor_copy(out=partial[:B, :], in_=psum_l)

        # ------------- logits exchange: 7 XOR-routed remote bcasts --------
        s1 = gpool.tile([B, E], F32)
        if USE_CC:
            dpool = ctx.enter_context(tc.tile_pool(name="dram", bufs=1, space="DRAM"))
            ar_in = dpool.tile([B, E], F32)
            nc.gpsimd.dma_start(out=ar_in, in_=partial[:B, :])
            ar_out = dpool.tile([B, E], F32, addr_space="Shared")
            nc.gpsimd.collective_compute(
                "AllReduce", AluOpType.add,
                replica_groups=[list(range(NCORES))],
                ins=[ar_in], outs=[ar_out],
            )
            nc.gpsimd.dma_start(out=s1, in_=ar_out)
        else:
            # self contribution into slot 0 (receiver j's slot r <- core j^r)
            nc.vector.tensor_copy(out=gath[:, 0, :], in_=partial)
            rsem = nc.alloc_semaphore("logit_rsem")
            lsem = nc.alloc_semaphore("logit_lsem")
            for r in range(1, NCORES):
                rd = [None] * 8
                rd[r] = (0, r)
                nc.gpsimd.remote_dma_broadcast(
                    out_ap=gath[:, r, :], in_ap=partial,
                    remote_sem=rsem, local_sem=lsem, rdests=rd,
                )
            trig = nc.gpsimd.trigger_dma(count=None)
            # sum slots: s1[b] = sum_r gath[b, r, :]
            s4 = gpool.tile([B, 4, E], F32)
            sum_inst = nc.vector.tensor_tensor(
                out=s4, in0=gath[:B, 0:4, :], in1=gath[:B, 4:8, :],
                op=AluOpType.add)
            s2 = gpool.tile([B, 2, E], F32)
            nc.vector.tensor_tensor(out=s2, in0=s4[:, 0:2, :],
                                    in1=s4[:, 2:4, :], op=AluOpType.add)
            nc.vector.tensor_tensor(out=s1, in0=s2[:, 0, :], in1=s2[:, 1, :],
                                    op=AluOpType.add)
            # The barrier wait (on trig) and the remote-arrival wait (on the
            # first gath reader) are satisfied by OTHER cores, which the
            # Tile scheduler's single-core sim cannot model — attached after
            # scheduling in make_nc().
            _POSTHOC["trig0"] = trig
            _POSTHOC["sum_waits"] = [(sum_inst, rsem, 2 * (NCORES - 1))]

        # ------------- select my batteries, top-2 gates -------------------
        logits_all = gpool.tile([B, E], F32)
        nc.vector.tensor_tensor(out=logits_all, in0=s1, in1=b2bc,
                                op=AluOpType.add)
        # project my 8 batteries to partitions 0..7
        psum_sel = gps.tile([BPC, E], F32, bufs=2, tag="pst")
        nc.tensor.matmul(out=psum_sel, lhsT=selt_sb, rhs=logits_all,
                         start=True, stop=True)
        logits_my = gpool.tile([BPC, E], F32)
        nc.vector.tensor_copy(out=logits_my, in_=psum_sel)

        sorted8 = gpool.tile([BPC, E], F32)
        sidx = gpool.tile([BPC, E], U32)
        nc.vector.max(out=sorted8, in_=logits_my)
        nc.vector.max_index(out=sidx, in_max=sorted8, in_values=logits_my)
        negmax = gpool.tile([BPC, 1], F32)
        nc.vector.tensor_scalar_mul(negmax, sorted8[:, 0:1], -1.0)
        q = gpool.tile([BPC, E], F32)
        nc.scalar.activation(out=q, in_=sorted8,
                             func=mybir.ActivationFunctionType.Exp,
                             bias=negmax, scale=1.0)
        zsum = gpool.tile([BPC, 1], F32)
        nc.vector.reduce_sum(out=zsum, in_=q, axis=mybir.AxisListType.X)
        t12 = gpool.tile([BPC, 1], F32)
        nc.vector.tensor_tensor(out=t12, in0=q[:, 0:1], in1=q[:, 1:2],
                                op=AluOpType.add)
        den = gpool.tile([BPC, 1], F32)
        nc.vector.scalar_tensor_tensor(out=den, in0=zsum, scalar=EPS, in1=t12,
                                       op0=AluOpType.mult, op1=AluOpType.add)
        rden = gpool.tile([BPC, 1], F32)
        nc.vector.reciprocal(out=rden, in_=den)
        g12 = gpool.tile([BPC, 2], F32)
        nc.vector.tensor_scalar_mul(g12, q[:, 0:2], rden)

        # broadcast gates to all partitions: [8,2] -fold-> [1,16] -mm-> [128,16]
        gflat = gpool.tile([1, 2 * BPC], F32)
        nc.gpsimd.dma_start(out=gflat, in_=g12)
        psum_bc = gps.tile([128, 2 * BPC], F32, bufs=2, tag="pst")
        nc.tensor.matmul(out=psum_bc, lhsT=onesrow, rhs=gflat,
                         start=True, stop=True)
        bcastG = gpool.tile([128, 2 * BPC], F32)
        nc.scalar.activation(out=bcastG, in_=psum_bc,
                             func=mybir.ActivationFunctionType.Copy)
        bcastGbf = gpool.tile([128, 2 * BPC], BF16)
        nc.vector.tensor_copy(out=bcastGbf, in_=psum_bc)
        gps_ctx.close()  # release gating PSUM banks

        mps = ctx.enter_context(tc.tile_pool(name="mpsum", bufs=8, space="PSUM"))

        # ------------- per-battery combine + fused matmuls ----------------
        def _vload(eng, ap, name):
            reg = eng.alloc_register(name)
            eng.reg_load(reg, ap)
            val = eng.snap(reg, donate=True)
            return nc.s_assert_within(val, 0, E - 1, skip_runtime_assert=True)

        wbs = {}

        def combine(b):
            """wb[kt] = g1*EXP[kt, e1] + g2*EXP[kt, e2], chunked per k-tile."""
            rv1 = _vload(nc.scalar, sidx[b:b + 1, 0:1], f"e1_{b}")
            rv2 = _vload(nc.vector, sidx[b:b + 1, 1:2], f"e2_{b}")
            wb = wbpool.tile([128, KT, D], BF16)
            for kt in range(KT):
                tsc = scpool.tile([128, D], BF16, tag="tsc", bufs=4)
                src1 = EXP_sb[:, kt, ds(rv1, 1), :].rearrange("p o d -> p (o d)")
                nc.scalar.activation(
                    out=tsc, in_=src1,
                    func=mybir.ActivationFunctionType.Copy,
                    scale=bcastG[:, 2 * b:2 * b + 1],
                )
                src2 = EXP_sb[:, kt, ds(rv2, 1), :].rearrange("p o d -> p (o d)")
                nc.vector.scalar_tensor_tensor(
                    out=wb[:, kt, :], in0=src2,
                    scalar=bcastGbf[:, 2 * b + 1:2 * b + 2],
                    in1=tsc, op0=AluOpType.mult, op1=AluOpType.add,
                )
            wbs[b] = wb

        def emit_battery(b):
            wb = wbs[b]
            psums = [mps.tile([128, D], F32, tag=f"pm{b % 2}_{m}", bufs=1,
                              name=f"pm_{b}_{m}")
                     for m in range(MT)]
            for kt in range(KT):
                for m in range(MT):
                    nc.tensor.matmul(
                        out=psums[m], lhsT=xt_sb[:, b, kt, ts(m, 128)],
                        rhs=wb[:, kt, :],
                        start=(kt == 0), stop=(kt == KT - 1),
                    )
            for m in range(MT):
                osb = opool.tile([128, D], BF16)
                nc.vector.tensor_copy(out=osb, in_=psums[m])
                nc.sync.dma_start(out=out_ap[b, m], in_=osb)

        combine(0)
        for b in range(BPC):
            if b + 1 < BPC:
                combine(b + 1)
            emit_battery(b)


def make_nc():
    from concourse import bacc
    nc = bacc.Bacc("TRN2", target_bir_lowering=False, debug=False,
                   num_devices=NCORES)
    build_program(nc)
    if not USE_CC:
        # Register the prelude AllGather (keeps NEFF collective init +
        # preamble barrier); nobody waits on its completion.  Attach each
        # ladder round's remote-arrival wait (2 increments per arrival) to
        # its local add; the Tile scheduler's single-core sim must not see
        # these waits (they cannot be satisfied locally).
        nc._bir_kernel_barrier_sem_replica_groups.append(set(range(NCORES)))
        if os.environ.get("MOE_BARRIER"):
            # gate the first send on the prelude AllGather completion
            _POSTHOC["trig0"].wait_op(
                nc._bir_kernel_barrier_sem, nc.bir_kernel_barrier_sem_inc,
                "sem-ge", check=False)
        for add, sem, val in _POSTHOC["sum_waits"]:
            add.wait_op(sem, val, "sem-ge", check=False)
    nc.finalize()
    return nc


def prep_inputs(cycle_curve_data, cycle_numbers, DKP_embeddings,
                gate_W1, gate_b1, gate_W2, gate_b2,
                expert_W, expert_b, gen_W, gen_b):
    """Host-side layout prep. Returns per-core in_maps list."""
    f32 = np.float32
    bf16 = ml_dtypes.bfloat16
    x = np.asarray(cycle_curve_data, dtype=f32).reshape(B, L, CF)

    # xt[b, p, kt, l] = xpad[b, kt*128+p, l]
    xpad = np.zeros((B, KP, L), dtype=bf16)
    xpad[:, :CF, :] = x.transpose(0, 2, 1).astype(bf16)
    xpad[:, CF, :] = np.asarray(1.0, dtype=bf16)
    xt_all = np.ascontiguousarray(
        xpad.reshape(B, KT, 128, L).transpose(0, 2, 1, 3))

    # fold general path into every expert (gates sum to 1)
    ew_f = np.asarray(expert_W, dtype=f32) + np.asarray(gen_W, dtype=f32)[None]
    eb_f = np.asarray(expert_b, dtype=f32) + np.asarray(gen_b, dtype=f32)[None]
    ew_p = np.zeros((E, KP, D), dtype=bf16)
    ew_p[:, :CF, :] = ew_f.astype(bf16)
    ew_p[:, CF, :] = eb_f.astype(bf16)
    # ew_dram[p, kt, e, d] = ew_p[e, kt*128+p, d]
    ew_dram = np.ascontiguousarray(
        ew_p.reshape(E, KT, 128, D).transpose(2, 1, 0, 3))

    gint = np.zeros((GK, B), dtype=f32)
    gint[:DLLM, :] = np.asarray(DKP_embeddings, dtype=f32).T
    gint[DLLM, :] = np.asarray(cycle_numbers, dtype=f32)[:, 0]
    gint[DLLM + 1, :] = 1.0
    gint_dram = np.ascontiguousarray(
        gint.reshape(GKT, 128, B).transpose(1, 0, 2))

    w1p = np.zeros((GK, DFF), dtype=f32)
    w1p[:DLLM + 1, :] = np.asarray(gate_W1, dtype=f32)
    w1p[DLLM + 1, :] = np.asarray(gate_b1, dtype=f32)

    w2 = np.asarray(gate_W2, dtype=f32)
    b2v = np.asarray(gate_b2, dtype=f32).reshape(1, E)

    in_maps = []
    for c in range(NCORES):
        w1slice = w1p[:, c * DFFC:(c + 1) * DFFC]
        w1_dram = np.ascontiguousarray(
            w1slice.reshape(GKT, 128, DFFC).transpose(1, 0, 2))
        w2slice = w2[c * DFFC:(c + 1) * DFFC, :]
        w2_dram = np.ascontiguousarray(
            w2slice.reshape(DFFC // 128, 128, E).transpose(1, 0, 2))
        sel = np.zeros((B, BPC), dtype=f32)
        for i in range(BPC):
            sel[c * BPC + i, i] = 1.0
        in_maps.append({
            "xt": np.ascontiguousarray(xt_all[c * BPC:(c + 1) * BPC]),
            "ew": ew_dram,
            "gint": gint_dram,
            "w1c": w1_dram,
            "w2c": w2_dram,
            "selt": sel,
            "b2": b2v,
        })
    return in_maps


_CACHED = {}


def run(inputs, trace=False, tmpdir=None):
    """Run on the 8 NeuronCores; returns (full_output, BassKernelResults)."""
    from concourse import bass_utils
    in_maps = prep_inputs(**inputs)
    nc = _CACHED.get("nc")
    if nc is None:
        nc = make_nc()
        _CACHED["nc"] = nc
    res = bass_utils.run_bass_kernel_spmd(
        nc, in_maps, core_ids=list(range(NCORES)), trace=trace, tmpdir=tmpdir
    )
    outs = [np.asarray(r["out"]).reshape(BPC, L, D) for r in res.results]
    full = np.concatenate(outs, axis=0).astype(np.float32)
    return full, res


def kernel(**inputs):
    full, _ = run(inputs, trace=False)
    return full


# revision 48
# speedup vs baseline: 1.4250x; 1.0140x over previous
"""Trainium2 Bass kernel for FlattenIntraCycleMoELayer (top-2 MoE + general path).

v2 strategy (see git/transcript for the trace analysis that motivated it):
  - Data-parallel over B (8 batteries per core).
  - gen_W is folded into every expert on the host (gates sum to 1), so each
    battery needs exactly one fused matmul pass: out = x @ (g1*We1' + g2*We2').
  - All DRAM layouts are partition-major so every bulk DMA is one
    contiguous-per-partition transfer (128 descriptors, full-rate HWDGE).
    Two HWDGE queues (SP=sync, ACT=scalar) stream: gating inputs first,
    then expert k-chunks, then per-battery x tiles.
  - Gating layer-1 runs in f32r (full fp32 operands, ~1 cyc/row at N=256),
    layer-2 in fp32; native Gelu activation.
  - Partial logits ([64,8] per core, d_ff-sharded) are exchanged with 7
    XOR-routed remote_dma_broadcast rounds (slot r of core j receives core
    j^r's partial; a sum is order-invariant) instead of a collective
    AllReduce; gated on the framework's bir-kernel barrier.
  - Top-2 select/softmax on a [8,E] tile, gates broadcast to 128 partitions
    via a K=1 matmul (ones ⊗ flat-gates), per-k-tile combine on
    scalar+vector chases the EXP DMA stream, fused matmuls run kt-outer so
    the first matmul needs only combine-chunk 0.
  - Output is evicted in bf16 and upcast on the host.

Host-side prep only reshapes/pads/casts/adds weight tensors (no forward-pass
math on host).
"""

import os
import numpy as np
import ml_dtypes


def _ensure_import_path():
    try:
        import concourse  # noqa: F401
    except ImportError:
        import sys
        for p in ("/opt/trn_rl_repo", "/root/.axon_site/_ro/trn_rl_repo"):
            if p not in sys.path:
                sys.path.insert(0, p)
        import concourse  # noqa: F401


_ensure_import_path()

import concourse.bass as bass  # noqa: E402
import concourse.tile as tile  # noqa: E402
from concourse import mybir  # noqa: E402
from concourse.bass import ds, ts  # noqa: E402
from concourse.alu_op_type import AluOpType  # noqa: E402
from concourse.masks import make_identity  # noqa: E402
from concourse.tile import add_dep_helper  # noqa: E402

BF16 = mybir.dt.bfloat16
F32 = mybir.dt.float32
U32 = mybir.dt.uint32
F32R = mybir.dt.float32r

# Problem shape constants (hardcoded per contest rules).
B, L, C, F = 64, 512, 3, 300
CF = C * F              # 900
KP = 1024               # padded contraction dim (900 data + 1 ones + 123 zero)
KT = KP // 128          # 8 k-tiles
D = 512                 # d_model
E = 8                   # experts
NCORES = 8
BPC = B // NCORES       # 8 batteries per core
DLLM = 4096
GK = 4224               # padded gating contraction = 33*128
GKT = GK // 128         # 33
DFF = 2048
DFFC = DFF // NCORES    # 256 per-core d_ff chunk
EPS = 1e-9
MT = L // 128           # 4 m-tiles per battery

USE_CC = bool(os.environ.get("MOE_CC"))   # fallback: collective AllReduce

_POSTHOC = {}   # instruction handles for waits attached after Tile scheduling


def build_program(nc):
    from contextlib import ExitStack

    xt = nc.dram_tensor("xt", [BPC, 128, KT, L], BF16, kind="ExternalInput")
    ew = nc.dram_tensor("ew", [128, KT, E, D], BF16, kind="ExternalInput")
    gint = nc.dram_tensor("gint", [128, GKT, B], F32R, kind="ExternalInput")
    w1c = nc.dram_tensor("w1c", [128, GKT, DFFC], F32R, kind="ExternalInput")
    w2c = nc.dram_tensor("w2c", [128, DFFC // 128, E], F32, kind="ExternalInput")
    selt = nc.dram_tensor("selt", [B, BPC], F32, kind="ExternalInput")
    b2 = nc.dram_tensor("b2", [1, E], F32, kind="ExternalInput")
    identd = nc.dram_tensor("identd", [128, 128], F32, kind="ExternalInput")
    onesd = nc.dram_tensor("onesd", [1, 128], F32, kind="ExternalInput")
    out = nc.dram_tensor("out", [BPC, MT, 128, D], BF16, kind="ExternalOutput")

    xt_ap = xt.ap()
    ew_ap = ew.ap()
    out_ap = out.ap()

    with tile.TileContext(nc) as tc, ExitStack() as ctx:
        singles = ctx.enter_context(tc.tile_pool(name="singles", bufs=1))
        gpool = ctx.enter_context(tc.tile_pool(name="gate", bufs=1))
        wbpool = ctx.enter_context(tc.tile_pool(name="wbs", bufs=2))
        scpool = ctx.enter_context(tc.tile_pool(name="scratch", bufs=2))
        opool = ctx.enter_context(tc.tile_pool(name="outs", bufs=4))
        gps_ctx = ExitStack()
        gps = gps_ctx.enter_context(tc.tile_pool(name="gpsum", bufs=1, space="PSUM"))
        w1_ctx = ExitStack()
        w1pool = w1_ctx.enter_context(tc.tile_pool(name="w1s", bufs=1))

        # ------------- DMA queue assignment ---------------------------------
        # scalar (ACT hwdge): ONLY two early W1 chunks — its ring waits must
        #   not head-of-line-block the gating/combine compute it runs later.
        # sync (SP hwdge): gint, the other W1 chunks, EXP k-chunks, xb0,
        #   then all output evictions.
        # gpsimd (sw dge): small tensors, the logits exchange, xb1..7.
        # gint first at full bandwidth (L1's lhsT), then the four W1 chunks
        # in parallel across both queues, then EXP k-chunks serially, with
        # xb0 alongside, then xb1..7 — all explicitly chained: HWDGE ring
        # entries execute concurrently, so priority requires dependencies.
        ginT_sb = w1pool.tile([128, GKT, B], F32R)
        gin_dma = nc.sync.dma_start(out=ginT_sb, in_=gint.ap())

        w1_sb = w1pool.tile([128, GKT, DFFC], F32R)
        w1ap = w1c.ap()
        w1_dmas = []
        for i, (a, b_, eng) in enumerate([(0, 9, nc.scalar), (9, 17, nc.sync),
                                          (17, 25, nc.scalar),
                                          (25, 33, nc.sync)]):
            dma = eng.dma_start(out=w1_sb[:, a:b_, :], in_=w1ap[:, a:b_, :])
            add_dep_helper(dma.ins, gin_dma.ins, sync=False,
                           reason="W1 after gint")
            w1_dmas.append(dma)

        EXP_sb = singles.tile([128, KT, E, D], BF16)
        xt_sb = singles.tile([128, BPC, KT, L], BF16)
        prev = None
        for kt in range(KT):
            dma = nc.sync.dma_start(out=EXP_sb[:, kt, :, :],
                                    in_=ew_ap[:, kt, :, :])
            if prev is None:
                for w in w1_dmas:
                    add_dep_helper(dma.ins, w.ins, sync=False,
                                   reason="EXP after W1")
            else:
                add_dep_helper(dma.ins, prev.ins, sync=False,
                               reason="EXP chain")
            prev = dma
            if kt == 0:
                xb0 = nc.sync.dma_start(out=xt_sb[:, 0, :, :], in_=xt_ap[0])
                for w in w1_dmas:
                    add_dep_helper(xb0.ins, w.ins, sync=False,
                                   reason="xb0 after W1")
        for b in range(1, BPC):
            dma = nc.sync.dma_start(out=xt_sb[:, b, :, :], in_=xt_ap[b])
            add_dep_helper(dma.ins, prev.ins, sync=False, reason="xb chain")
            prev = dma

        # small gating tensors on gpsimd (software DGE queue)
        w2_sb = gpool.tile([128, DFFC // 128, E], F32)
        nc.gpsimd.dma_start(out=w2_sb, in_=w2c.ap())
        selt_sb = gpool.tile([B, BPC], F32)
        nc.gpsimd.dma_start(out=selt_sb, in_=selt.ap())
        b2_ap = b2.ap()
        b2bc = gpool.tile([B, E], F32)
        nc.gpsimd.dma_start(
            out=b2bc,
            in_=bass.AP(tensor=b2_ap.tensor, offset=b2_ap.offset,
                        ap=[[0, B]] + list(b2_ap.ap[1:])),
        )
        # identity/ones come from host inputs and partial is zeroed on the
        # DVE: the gpsimd engine must never run pool-library ops, or the
        # ucode library swap delays the swdge logits exchange by ~20us.
        ident = singles.tile([128, 128], F32)
        nc.gpsimd.dma_start(out=ident, in_=identd.ap())
        onesrow = singles.tile([1, 128], F32)
        nc.gpsimd.dma_start(out=onesrow, in_=onesd.ap())

        # exchange buffers; partial's pad partitions are zeroed (they are
        # broadcast to peers).  gath is never memset — remote writes from
        # peers land in it and a local memset could race them.
        partial = gpool.tile([128, E], F32)
        nc.vector.memset(partial, 0.0)
        gath = gpool.tile([128, NCORES, E], F32)

        # ------------- gating layer 1 (f32r), chasing the W1 stream -------
        psum_h = gps.tile([B, DFFC], F32, bufs=1)
        for kt in range(GKT):
            nc.tensor.matmul(
                out=psum_h,
                lhsT=ginT_sb[:, kt, :],
                rhs=w1_sb[:, kt, :],
                start=(kt == 0), stop=(kt == GKT - 1),
            )
        w1_ctx.close()  # release W1 SBUF
        # gelu, tanh approx (matches jax.nn.gelu default):
        #   h = 0.5*x*(1 + tanh(0.79788456*(x + 0.044715*x^3)))
        g_x = gpool.tile([B, DFFC], F32)
        nc.vector.tensor_copy(out=g_x, in_=psum_h)
        g_x2 = gpool.tile([B, DFFC], F32)
        nc.vector.tensor_tensor(out=g_x2, in0=g_x, in1=g_x, op=AluOpType.mult)
        g_p = gpool.tile([B, DFFC], F32)
        nc.vector.tensor_scalar(g_p, g_x2, 0.044715, 1.0,
                                AluOpType.mult, AluOpType.add)
        g_u = gpool.tile([B, DFFC], F32)
        nc.vector.tensor_tensor(out=g_u, in0=g_x, in1=g_p, op=AluOpType.mult)
        g_t = gpool.tile([B, DFFC], F32)
        nc.scalar.activation(out=g_t, in_=g_u,
                             func=mybir.ActivationFunctionType.Tanh,
                             scale=0.7978845608028654)
        g_q = gpool.tile([B, DFFC], F32)
        nc.vector.tensor_scalar(g_q, g_t, 1.0, 0.5,
                                AluOpType.add, AluOpType.mult)
        h_sb = gpool.tile([B, DFFC], F32)
        nc.vector.tensor_tensor(out=h_sb, in0=g_x, in1=g_q, op=AluOpType.mult)
        # transpose h -> hT [128, 2, B]
        hT_sb = gpool.tile([128, DFFC // 128, B], F32)
        for j in range(DFFC // 128):
            pst = gps.tile([128, B], F32, bufs=2, tag="pst")
            nc.tensor.transpose(
                out=pst, in_=h_sb[:, j * 128:(j + 1) * 128], identity=ident[:B, :B]
            )
            nc.vector.tensor_copy(out=hT_sb[:, j, :], in_=pst)
        # layer 2 partial logits [B, E]
        psum_l = gps.tile([B, E], F32, bufs=2, tag="pst")
        for j in range(DFFC // 128):
            nc.tensor.matmul(
                out=psum_l, lhsT=hT_sb[:, j, :], rhs=w2_sb[:, j, :],
                start=(j == 0), stop=(j == DFFC // 128 - 1),
            )
        nc.vector.tensor_copy(out=partial[:B, :], in_=psum_l)

        # ------------- logits exchange: 7 XOR-routed remote bcasts --------
        s1 = gpool.tile([B, E], F32)
        if USE_CC:
            dpool = ctx.enter_context(tc.tile_pool(name="dram", bufs=1, space="DRAM"))
            ar_in = dpool.tile([B, E], F32)
            nc.gpsimd.dma_start(out=ar_in, in_=partial[:B, :])
            ar_out = dpool.tile([B, E], F32, addr_space="Shared")
            nc.gpsimd.collective_compute(
                "AllReduce", AluOpType.add,
                replica_groups=[list(range(NCORES))],
                ins=[ar_in], outs=[ar_out],
            )
            nc.gpsimd.dma_start(out=s1, in_=ar_out)
        else:
            # self contribution into slot 0 (receiver j's slot r <- core j^r)
            nc.vector.tensor_copy(out=gath[:, 0, :], in_=partial)
            rsem = nc.alloc_semaphore("logit_rsem")
            lsem = nc.alloc_semaphore("logit_lsem")
            for r in range(1, NCORES):
                rd = [None] * 8
                rd[r] = (0, r)
                nc.gpsimd.remote_dma_broadcast(
                    out_ap=gath[:, r, :], in_ap=partial,
                    remote_sem=rsem, local_sem=lsem, rdests=rd,
                )
            trig = nc.gpsimd.trigger_dma(count=None)
            # sum slots: s1[b] = sum_r gath[b, r, :]
            s4 = gpool.tile([B, 4, E], F32)
            sum_inst = nc.vector.tensor_tensor(
                out=s4, in0=gath[:B, 0:4, :], in1=gath[:B, 4:8, :],
                op=AluOpType.add)
            s2 = gpool.tile([B, 2, E], F32)
            nc.vector.tensor_tensor(out=s2, in0=s4[:, 0:2, :],
                                    in1=s4[:, 2:4, :], op=AluOpType.add)
            nc.vector.tensor_tensor(out=s1, in0=s2[:, 0, :], in1=s2[:, 1, :],
                                    op=AluOpType.add)
            # The barrier wait (on trig) and the remote-arrival wait (on the
            # first gath reader) are satisfied by OTHER cores, which the
            # Tile scheduler's single-core sim cannot model — attached after
            # scheduling in make_nc().
            _POSTHOC["trig0"] = trig
            _POSTHOC["sum_waits"] = [(sum_inst, rsem, 2 * (NCORES - 1))]

        # ------------- select my batteries, top-2 gates -------------------
        logits_all = gpool.tile([B, E], F32)
        nc.vector.tensor_tensor(out=logits_all, in0=s1, in1=b2bc,
                                op=AluOpType.add)
        # project my 8 batteries to partitions 0..7
        psum_sel = gps.tile([BPC, E], F32, bufs=2, tag="pst")
        nc.tensor.matmul(out=psum_sel, lhsT=selt_sb, rhs=logits_all,
                         start=True, stop=True)
        logits_my = gpool.tile([BPC, E], F32)
        nc.vector.tensor_copy(out=logits_my, in_=psum_sel)

        sorted8 = gpool.tile([BPC, E], F32)
        sidx = gpool.tile([BPC, E], U32)
        nc.vector.max(out=sorted8, in_=logits_my)
        nc.vector.max_index(out=sidx, in_max=sorted8, in_values=logits_my)
        negmax = gpool.tile([BPC, 1], F32)
        nc.vector.tensor_scalar_mul(negmax, sorted8[:, 0:1], -1.0)
        q = gpool.tile([BPC, E], F32)
        nc.scalar.activation(out=q, in_=sorted8,
                             func=mybir.ActivationFunctionType.Exp,
                             bias=negmax, scale=1.0)
        zsum = gpool.tile([BPC, 1], F32)
        nc.vector.reduce_sum(out=zsum, in_=q, axis=mybir.AxisListType.X)
        t12 = gpool.tile([BPC, 1], F32)
        nc.vector.tensor_tensor(out=t12, in0=q[:, 0:1], in1=q[:, 1:2],
                                op=AluOpType.add)
        den = gpool.tile([BPC, 1], F32)
        nc.vector.scalar_tensor_tensor(out=den, in0=zsum, scalar=EPS, in1=t12,
                                       op0=AluOpType.mult, op1=AluOpType.add)
        rden = gpool.tile([BPC, 1], F32)
        nc.vector.reciprocal(out=rden, in_=den)
        g12 = gpool.tile([BPC, 2], F32)
        nc.vector.tensor_scalar_mul(g12, q[:, 0:2], rden)

        # broadcast gates to all partitions: [8,2] -fold-> [1,16] -mm-> [128,16]
        gflat = gpool.tile([1, 2 * BPC], F32)
        nc.gpsimd.dma_start(out=gflat, in_=g12)
        psum_bc = gps.tile([128, 2 * BPC], F32, bufs=2, tag="pst")
        nc.tensor.matmul(out=psum_bc, lhsT=onesrow, rhs=gflat,
                         start=True, stop=True)
        bcastG = gpool.tile([128, 2 * BPC], F32)
        nc.scalar.activation(out=bcastG, in_=psum_bc,
                             func=mybir.ActivationFunctionType.Copy)
        bcastGbf = gpool.tile([128, 2 * BPC], BF16)
        nc.vector.tensor_copy(out=bcastGbf, in_=psum_bc)
        gps_ctx.close()  # release gating PSUM banks

        mps = ctx.enter_context(tc.tile_pool(name="mpsum", bufs=8, space="PSUM"))

        # ------------- per-battery combine + fused matmuls ----------------
        def _vload(eng, ap, name):
            reg = eng.alloc_register(name)
            eng.reg_load(reg, ap)
            val = eng.snap(reg, donate=True)
            return nc.s_assert_within(val, 0, E - 1, skip_runtime_assert=True)

        wbs = {}

        def combine(b):
            """wb[kt] = g1*EXP[kt, e1] + g2*EXP[kt, e2], chunked per k-tile."""
            rv1 = _vload(nc.scalar, sidx[b:b + 1, 0:1], f"e1_{b}")
            rv2 = _vload(nc.vector, sidx[b:b + 1, 1:2], f"e2_{b}")
            wb = wbpool.tile([128, KT, D], BF16)
            for kt in range(KT):
                tsc = scpool.tile([128, D], BF16, tag="tsc", bufs=4)
                src1 = EXP_sb[:, kt, ds(rv1, 1), :].rearrange("p o d -> p (o d)")
                nc.scalar.activation(
                    out=tsc, in_=src1,
                    func=mybir.ActivationFunctionType.Copy,
                    scale=bcastG[:, 2 * b:2 * b + 1],
                )
                src2 = EXP_sb[:, kt, ds(rv2, 1), :].rearrange("p o d -> p (o d)")
                nc.vector.scalar_tensor_tensor(
                    out=wb[:, kt, :], in0=src2,
                    scalar=bcastGbf[:, 2 * b + 1:2 * b + 2],
                    in1=tsc, op0=AluOpType.mult, op1=AluOpType.add,
                )
            wbs[b] = wb

        def emit_battery(b):
            wb = wbs[b]
            psums = [mps.tile([128, D], F32, tag=f"pm{b % 2}_{m}", bufs=1,
                              name=f"pm_{b}_{m}")
                     for m in range(MT)]
            for kt in range(KT):
                for m in range(MT):
                    nc.tensor.matmul(
                        out=psums[m], lhsT=xt_sb[:, b, kt, ts(m, 128)],
                        rhs=wb[:, kt, :],
                        start=(kt == 0), stop=(kt == KT - 1),
                    )
            for m in range(MT):
                osb = opool.tile([128, D], BF16)
                nc.vector.tensor_copy(out=osb, in_=psums[m])
                nc.sync.dma_start(out=out_ap[b, m], in_=osb)

        combine(0)
        for b in range(BPC):
            if b + 1 < BPC:
                combine(b + 1)
            emit_battery(b)


def make_nc():
    from concourse import bacc
    nc = bacc.Bacc("TRN2", target_bir_lowering=False, debug=False,
                   num_devices=NCORES)
    build_program(nc)
    if not USE_CC:
        # Register the prelude AllGather (keeps NEFF collective init +
        # preamble barrier); nobody waits on its completion.  Attach each
        # ladder round's remote-arrival wait (2 increments per arrival) to
        # its local add; the Tile scheduler's single-core sim must not see
        # these waits (they cannot be satisfied locally).
        nc._bir_kernel_barrier_sem_replica_groups.append(set(range(NCORES)))
        if os.environ.get("MOE_BARRIER"):
            # gate the first send on the prelude AllGather completion
            _POSTHOC["trig0"].wait_op(
                nc._bir_kernel_barrier_sem, nc.bir_kernel_barrier_sem_inc,
                "sem-ge", check=False)
        for add, sem, val in _POSTHOC["sum_waits"]:
            add.wait_op(sem, val, "sem-ge", check=False)
    nc.finalize()
    return nc


def prep_inputs(cycle_curve_data, cycle_numbers, DKP_embeddings,
                gate_W1, gate_b1, gate_W2, gate_b2,
                expert_W, expert_b, gen_W, gen_b):
    """Host-side layout prep. Returns per-core in_maps list."""
    f32 = np.float32
    bf16 = ml_dtypes.bfloat16
    x = np.asarray(cycle_curve_data, dtype=f32).reshape(B, L, CF)

    # xt[b, p, kt, l] = xpad[b, kt*128+p, l]
    xpad = np.zeros((B, KP, L), dtype=bf16)
    xpad[:, :CF, :] = x.transpose(0, 2, 1).astype(bf16)
    xpad[:, CF, :] = np.asarray(1.0, dtype=bf16)
    xt_all = np.ascontiguousarray(
        xpad.reshape(B, KT, 128, L).transpose(0, 2, 1, 3))

    # fold general path into every expert (gates sum to 1)
    ew_f = np.asarray(expert_W, dtype=f32) + np.asarray(gen_W, dtype=f32)[None]
    eb_f = np.asarray(expert_b, dtype=f32) + np.asarray(gen_b, dtype=f32)[None]
    ew_p = np.zeros((E, KP, D), dtype=bf16)
    ew_p[:, :CF, :] = ew_f.astype(bf16)
    ew_p[:, CF, :] = eb_f.astype(bf16)
    # ew_dram[p, kt, e, d] = ew_p[e, kt*128+p, d]
    ew_dram = np.ascontiguousarray(
        ew_p.reshape(E, KT, 128, D).transpose(2, 1, 0, 3))

    gint = np.zeros((GK, B), dtype=f32)
    gint[:DLLM, :] = np.asarray(DKP_embeddings, dtype=f32).T
    gint[DLLM, :] = np.asarray(cycle_numbers, dtype=f32)[:, 0]
    gint[DLLM + 1, :] = 1.0
    gint_dram = np.ascontiguousarray(
        gint.reshape(GKT, 128, B).transpose(1, 0, 2))

    w1p = np.zeros((GK, DFF), dtype=f32)
    w1p[:DLLM + 1, :] = np.asarray(gate_W1, dtype=f32)
    w1p[DLLM + 1, :] = np.asarray(gate_b1, dtype=f32)

    w2 = np.asarray(gate_W2, dtype=f32)
    b2v = np.asarray(gate_b2, dtype=f32).reshape(1, E)

    in_maps = []
    for c in range(NCORES):
        w1slice = w1p[:, c * DFFC:(c + 1) * DFFC]
        w1_dram = np.ascontiguousarray(
            w1slice.reshape(GKT, 128, DFFC).transpose(1, 0, 2))
        w2slice = w2[c * DFFC:(c + 1) * DFFC, :]
        w2_dram = np.ascontiguousarray(
            w2slice.reshape(DFFC // 128, 128, E).transpose(1, 0, 2))
        sel = np.zeros((B, BPC), dtype=f32)
        for i in range(BPC):
            sel[c * BPC + i, i] = 1.0
        in_maps.append({
            "xt": np.ascontiguousarray(xt_all[c * BPC:(c + 1) * BPC]),
            "ew": ew_dram,
            "gint": gint_dram,
            "w1c": w1_dram,
            "w2c": w2_dram,
            "selt": sel,
            "b2": b2v,
            "identd": np.eye(128, dtype=f32),
            "onesd": np.ones((1, 128), dtype=f32),
        })
    return in_maps


_CACHED = {}


def run(inputs, trace=False, tmpdir=None):
    """Run on the 8 NeuronCores; returns (full_output, BassKernelResults)."""
    from concourse import bass_utils
    in_maps = prep_inputs(**inputs)
    nc = _CACHED.get("nc")
    if nc is None:
        nc = make_nc()
        _CACHED["nc"] = nc
    res = bass_utils.run_bass_kernel_spmd(
        nc, in_maps, core_ids=list(range(NCORES)), trace=trace, tmpdir=tmpdir
    )
    outs = [np.asarray(r["out"]).reshape(BPC, L, D) for r in res.results]
    full = np.concatenate(outs, axis=0).astype(np.float32)
    return full, res


def kernel(**inputs):
    full, _ = run(inputs, trace=False)
    return full


# revision 50
# speedup vs baseline: 1.5498x; 1.0876x over previous
"""Trainium2 Bass kernel for FlattenIntraCycleMoELayer (top-2 MoE + general path).

v2 strategy (see git/transcript for the trace analysis that motivated it):
  - Data-parallel over B (8 batteries per core).
  - gen_W is folded into every expert on the host (gates sum to 1), so each
    battery needs exactly one fused matmul pass: out = x @ (g1*We1' + g2*We2').
  - All DRAM layouts are partition-major so every bulk DMA is one
    contiguous-per-partition transfer (128 descriptors, full-rate HWDGE).
    Two HWDGE queues (SP=sync, ACT=scalar) stream: gating inputs first,
    then expert k-chunks, then per-battery x tiles.
  - Gating layer-1 runs in f32r (full fp32 operands, ~1 cyc/row at N=256),
    layer-2 in fp32; native Gelu activation.
  - Partial logits ([64,8] per core, d_ff-sharded) are exchanged with 7
    XOR-routed remote_dma_broadcast rounds (slot r of core j receives core
    j^r's partial; a sum is order-invariant) instead of a collective
    AllReduce; gated on the framework's bir-kernel barrier.
  - Top-2 select/softmax on a [8,E] tile, gates broadcast to 128 partitions
    via a K=1 matmul (ones ⊗ flat-gates), per-k-tile combine on
    scalar+vector chases the EXP DMA stream, fused matmuls run kt-outer so
    the first matmul needs only combine-chunk 0.
  - Output is evicted in bf16 and upcast on the host.

Host-side prep only reshapes/pads/casts/adds weight tensors (no forward-pass
math on host).
"""

import os
import numpy as np
import ml_dtypes


def _ensure_import_path():
    try:
        import concourse  # noqa: F401
    except ImportError:
        import sys
        for p in ("/opt/trn_rl_repo", "/root/.axon_site/_ro/trn_rl_repo"):
            if p not in sys.path:
                sys.path.insert(0, p)
        import concourse  # noqa: F401


_ensure_import_path()

import concourse.bass as bass  # noqa: E402
import concourse.tile as tile  # noqa: E402
from concourse import mybir  # noqa: E402
from concourse.bass import ds, ts  # noqa: E402
from concourse.alu_op_type import AluOpType  # noqa: E402
from concourse.masks import make_identity  # noqa: E402
from concourse.tile import add_dep_helper  # noqa: E402

BF16 = mybir.dt.bfloat16
F32 = mybir.dt.float32
U32 = mybir.dt.uint32
F32R = mybir.dt.float32r

# Problem shape constants (hardcoded per contest rules).
B, L, C, F = 64, 512, 3, 300
CF = C * F              # 900
KP = 1024               # padded contraction dim (900 data + 1 ones + 123 zero)
KT = KP // 128          # 8 k-tiles
D = 512                 # d_model
E = 8                   # experts
NCORES = 8
BPC = B // NCORES       # 8 batteries per core
DLLM = 4096
GK = 4224               # padded gating contraction = 33*128
GKT = GK // 128         # 33
DFF = 2048
DFFC = DFF // NCORES    # 256 per-core d_ff chunk
EPS = 1e-9
MT = L // 128           # 4 m-tiles per battery

USE_CC = bool(os.environ.get("MOE_CC"))   # fallback: collective AllReduce

_POSTHOC = {}   # instruction handles for waits attached after Tile scheduling


def build_program(nc):
    from contextlib import ExitStack

    xt = nc.dram_tensor("xt", [BPC, 128, KT, L], BF16, kind="ExternalInput")
    ew = nc.dram_tensor("ew", [128, KT, E, D], BF16, kind="ExternalInput")
    gint = nc.dram_tensor("gint", [128, GKT, B], F32R, kind="ExternalInput")
    w1c = nc.dram_tensor("w1c", [128, GKT, DFFC], F32R, kind="ExternalInput")
    w2c = nc.dram_tensor("w2c", [128, DFFC // 128, E], F32, kind="ExternalInput")
    selt = nc.dram_tensor("selt", [B, BPC], F32, kind="ExternalInput")
    b2 = nc.dram_tensor("b2", [1, E], F32, kind="ExternalInput")
    identd = nc.dram_tensor("identd", [128, 128], F32, kind="ExternalInput")
    onesd = nc.dram_tensor("onesd", [1, 128], F32, kind="ExternalInput")
    out = nc.dram_tensor("out", [BPC, MT, 128, D], BF16, kind="ExternalOutput")

    xt_ap = xt.ap()
    ew_ap = ew.ap()
    out_ap = out.ap()

    with tile.TileContext(nc) as tc, ExitStack() as ctx:
        singles = ctx.enter_context(tc.tile_pool(name="singles", bufs=1))
        gpool = ctx.enter_context(tc.tile_pool(name="gate", bufs=1))
        wbpool = ctx.enter_context(tc.tile_pool(name="wbs", bufs=2))
        scpool = ctx.enter_context(tc.tile_pool(name="scratch", bufs=2))
        opool = ctx.enter_context(tc.tile_pool(name="outs", bufs=4))
        gps_ctx = ExitStack()
        gps = gps_ctx.enter_context(tc.tile_pool(name="gpsum", bufs=1, space="PSUM"))
        w1_ctx = ExitStack()
        w1pool = w1_ctx.enter_context(tc.tile_pool(name="w1s", bufs=1))

        # ------------- DMA queue assignment ---------------------------------
        # scalar (ACT hwdge): ONLY two early W1 chunks — its ring waits must
        #   not head-of-line-block the gating/combine compute it runs later.
        # sync (SP hwdge): gint, the other W1 chunks, EXP k-chunks, xb0,
        #   then all output evictions.
        # gpsimd (sw dge): small tensors, the logits exchange, xb1..7.
        # gint first at full bandwidth (L1's lhsT), then the four W1 chunks
        # in parallel across both queues, then EXP k-chunks serially, with
        # xb0 alongside, then xb1..7 — all explicitly chained: HWDGE ring
        # entries execute concurrently, so priority requires dependencies.
        ginT_sb = w1pool.tile([128, GKT, B], F32R)
        gin_dma = nc.sync.dma_start(out=ginT_sb, in_=gint.ap())

        w1_sb = w1pool.tile([128, GKT, DFFC], F32R)
        w1ap = w1c.ap()
        w1_dmas = []
        for i, (a, b_, eng) in enumerate([(0, 9, nc.scalar), (9, 17, nc.sync),
                                          (17, 25, nc.scalar),
                                          (25, 33, nc.sync)]):
            dma = eng.dma_start(out=w1_sb[:, a:b_, :], in_=w1ap[:, a:b_, :])
            add_dep_helper(dma.ins, gin_dma.ins, sync=False,
                           reason="W1 after gint")
            w1_dmas.append(dma)

        EXP_sb = singles.tile([128, KT, E, D], BF16)
        xt_sb = singles.tile([128, BPC, KT, L], BF16)
        prev = None
        for kt in range(KT):
            dma = nc.sync.dma_start(out=EXP_sb[:, kt, :, :],
                                    in_=ew_ap[:, kt, :, :])
            if prev is None:
                for w in w1_dmas:
                    add_dep_helper(dma.ins, w.ins, sync=False,
                                   reason="EXP after W1")
            else:
                add_dep_helper(dma.ins, prev.ins, sync=False,
                               reason="EXP chain")
            prev = dma
            if kt == 0:
                xb0 = nc.sync.dma_start(out=xt_sb[:, 0, :, :], in_=xt_ap[0])
                for w in w1_dmas:
                    add_dep_helper(xb0.ins, w.ins, sync=False,
                                   reason="xb0 after W1")
        for b in range(1, BPC):
            dma = nc.sync.dma_start(out=xt_sb[:, b, :, :], in_=xt_ap[b])
            add_dep_helper(dma.ins, prev.ins, sync=False, reason="xb chain")
            prev = dma

        # small gating tensors on gpsimd (software DGE queue)
        w2_sb = gpool.tile([128, DFFC // 128, E], F32)
        nc.gpsimd.dma_start(out=w2_sb, in_=w2c.ap())
        selt_sb = gpool.tile([B, BPC], F32)
        nc.gpsimd.dma_start(out=selt_sb, in_=selt.ap())
        b2_ap = b2.ap()
        b2bc = gpool.tile([B, E], F32)
        nc.gpsimd.dma_start(
            out=b2bc,
            in_=bass.AP(tensor=b2_ap.tensor, offset=b2_ap.offset,
                        ap=[[0, B]] + list(b2_ap.ap[1:])),
        )
        # identity/ones come from host inputs and partial is zeroed on the
        # DVE: the gpsimd engine must never run pool-library ops, or the
        # ucode library swap delays the swdge logits exchange by ~20us.
        ident = singles.tile([128, 128], F32)
        nc.gpsimd.dma_start(out=ident, in_=identd.ap())
        onesrow = singles.tile([1, 128], F32)
        nc.gpsimd.dma_start(out=onesrow, in_=onesd.ap())

        # exchange buffers; partial's pad partitions are zeroed (they are
        # broadcast to peers).  gath is never memset — remote writes from
        # peers land in it and a local memset could race them.
        partial = gpool.tile([128, E], F32)
        nc.vector.memset(partial, 0.0)
        gath = gpool.tile([128, NCORES, E], F32)

        # ------------- gating layer 1 (f32r), chasing the W1 stream -------
        psum_h = gps.tile([B, DFFC], F32, bufs=1)
        for kt in range(GKT):
            nc.tensor.matmul(
                out=psum_h,
                lhsT=ginT_sb[:, kt, :],
                rhs=w1_sb[:, kt, :],
                start=(kt == 0), stop=(kt == GKT - 1),
            )
        w1_ctx.close()  # release W1 SBUF
        # gelu, tanh approx (matches jax.nn.gelu default):
        #   h = 0.5*x*(1 + tanh(0.79788456*(x + 0.044715*x^3)))
        g_x = gpool.tile([B, DFFC], F32)
        nc.vector.tensor_copy(out=g_x, in_=psum_h)
        g_x2 = gpool.tile([B, DFFC], F32)
        nc.vector.tensor_tensor(out=g_x2, in0=g_x, in1=g_x, op=AluOpType.mult)
        g_p = gpool.tile([B, DFFC], F32)
        nc.vector.tensor_scalar(g_p, g_x2, 0.044715, 1.0,
                                AluOpType.mult, AluOpType.add)
        g_u = gpool.tile([B, DFFC], F32)
        nc.vector.tensor_tensor(out=g_u, in0=g_x, in1=g_p, op=AluOpType.mult)
        g_t = gpool.tile([B, DFFC], F32)
        nc.scalar.activation(out=g_t, in_=g_u,
                             func=mybir.ActivationFunctionType.Tanh,
                             scale=0.7978845608028654)
        g_q = gpool.tile([B, DFFC], F32)
        nc.vector.tensor_scalar(g_q, g_t, 1.0, 0.5,
                                AluOpType.add, AluOpType.mult)
        h_sb = gpool.tile([B, DFFC], F32)
        nc.vector.tensor_tensor(out=h_sb, in0=g_x, in1=g_q, op=AluOpType.mult)
        # transpose h -> hT [128, 2, B]
        hT_sb = gpool.tile([128, DFFC // 128, B], F32)
        for j in range(DFFC // 128):
            pst = gps.tile([128, B], F32, bufs=2, tag="pst")
            nc.tensor.transpose(
                out=pst, in_=h_sb[:, j * 128:(j + 1) * 128], identity=ident[:B, :B]
            )
            nc.vector.tensor_copy(out=hT_sb[:, j, :], in_=pst)
        # layer 2 partial logits [B, E]
        psum_l = gps.tile([B, E], F32, bufs=2, tag="pst")
        for j in range(DFFC // 128):
            nc.tensor.matmul(
                out=psum_l, lhsT=hT_sb[:, j, :], rhs=w2_sb[:, j, :],
                start=(j == 0), stop=(j == DFFC // 128 - 1),
            )
        nc.vector.tensor_copy(out=partial[:B, :], in_=psum_l)

        # ------------- logits exchange: 7 XOR-routed remote bcasts --------
        s1 = gpool.tile([B, E], F32)
        if USE_CC:
            dpool = ctx.enter_context(tc.tile_pool(name="dram", bufs=1, space="DRAM"))
            ar_in = dpool.tile([B, E], F32)
            nc.gpsimd.dma_start(out=ar_in, in_=partial[:B, :])
            ar_out = dpool.tile([B, E], F32, addr_space="Shared")
            nc.gpsimd.collective_compute(
                "AllReduce", AluOpType.add,
                replica_groups=[list(range(NCORES))],
                ins=[ar_in], outs=[ar_out],
            )
            nc.gpsimd.dma_start(out=s1, in_=ar_out)
        else:
            # self contribution into slot 0 (receiver j's slot r <- core j^r)
            nc.vector.tensor_copy(out=gath[:, 0, :], in_=partial)
            rsem = nc.alloc_semaphore("logit_rsem")
            lsem = nc.alloc_semaphore("logit_lsem")
            # The swdge ring processes remote descriptors serially (~48ns
            # each, 128 per round); spread the 7 rounds over all 4 swdge
            # queues — one trigger per queue (multiple triggers on one queue
            # hang the ucode) — so the rings drain in parallel.
            qof = {1: 0, 2: 0, 3: 1, 4: 1, 5: 2, 6: 2, 7: 3}
            for r in range(1, NCORES):
                rd = [None] * 8
                rd[r] = (0, r)
                nc.gpsimd.remote_dma_broadcast(
                    out_ap=gath[:, r, :], in_ap=partial,
                    remote_sem=rsem, local_sem=lsem, rdests=rd,
                    queue_num=qof[r],
                )
            trig = None
            for q in range(4):
                t = nc.gpsimd.trigger_dma(count=None, queue_num=q)
                trig = trig or t
            # sum slots: s1[b] = sum_r gath[b, r, :]
            s4 = gpool.tile([B, 4, E], F32)
            sum_inst = nc.vector.tensor_tensor(
                out=s4, in0=gath[:B, 0:4, :], in1=gath[:B, 4:8, :],
                op=AluOpType.add)
            s2 = gpool.tile([B, 2, E], F32)
            nc.vector.tensor_tensor(out=s2, in0=s4[:, 0:2, :],
                                    in1=s4[:, 2:4, :], op=AluOpType.add)
            nc.vector.tensor_tensor(out=s1, in0=s2[:, 0, :], in1=s2[:, 1, :],
                                    op=AluOpType.add)
            # The barrier wait (on trig) and the remote-arrival wait (on the
            # first gath reader) are satisfied by OTHER cores, which the
            # Tile scheduler's single-core sim cannot model — attached after
            # scheduling in make_nc().
            _POSTHOC["trig0"] = trig
            _POSTHOC["sum_waits"] = [(sum_inst, rsem, 2 * (NCORES - 1))]

        # ------------- select my batteries, top-2 gates -------------------
        logits_all = gpool.tile([B, E], F32)
        nc.vector.tensor_tensor(out=logits_all, in0=s1, in1=b2bc,
                                op=AluOpType.add)
        # project my 8 batteries to partitions 0..7
        psum_sel = gps.tile([BPC, E], F32, bufs=2, tag="pst")
        nc.tensor.matmul(out=psum_sel, lhsT=selt_sb, rhs=logits_all,
                         start=True, stop=True)
        logits_my = gpool.tile([BPC, E], F32)
        nc.vector.tensor_copy(out=logits_my, in_=psum_sel)

        sorted8 = gpool.tile([BPC, E], F32)
        sidx = gpool.tile([BPC, E], U32)
        nc.vector.max(out=sorted8, in_=logits_my)
        nc.vector.max_index(out=sidx, in_max=sorted8, in_values=logits_my)
        negmax = gpool.tile([BPC, 1], F32)
        nc.vector.tensor_scalar_mul(negmax, sorted8[:, 0:1], -1.0)
        q = gpool.tile([BPC, E], F32)
        nc.scalar.activation(out=q, in_=sorted8,
                             func=mybir.ActivationFunctionType.Exp,
                             bias=negmax, scale=1.0)
        zsum = gpool.tile([BPC, 1], F32)
        nc.vector.reduce_sum(out=zsum, in_=q, axis=mybir.AxisListType.X)
        t12 = gpool.tile([BPC, 1], F32)
        nc.vector.tensor_tensor(out=t12, in0=q[:, 0:1], in1=q[:, 1:2],
                                op=AluOpType.add)
        den = gpool.tile([BPC, 1], F32)
        nc.vector.scalar_tensor_tensor(out=den, in0=zsum, scalar=EPS, in1=t12,
                                       op0=AluOpType.mult, op1=AluOpType.add)
        rden = gpool.tile([BPC, 1], F32)
        nc.vector.reciprocal(out=rden, in_=den)
        g12 = gpool.tile([BPC, 2], F32)
        nc.vector.tensor_scalar_mul(g12, q[:, 0:2], rden)

        # broadcast gates to all partitions: [8,2] -fold-> [1,16] -mm-> [128,16]
        gflat = gpool.tile([1, 2 * BPC], F32)
        nc.gpsimd.dma_start(out=gflat, in_=g12)
        psum_bc = gps.tile([128, 2 * BPC], F32, bufs=2, tag="pst")
        nc.tensor.matmul(out=psum_bc, lhsT=onesrow, rhs=gflat,
                         start=True, stop=True)
        bcastG = gpool.tile([128, 2 * BPC], F32)
        nc.scalar.activation(out=bcastG, in_=psum_bc,
                             func=mybir.ActivationFunctionType.Copy)
        bcastGbf = gpool.tile([128, 2 * BPC], BF16)
        nc.vector.tensor_copy(out=bcastGbf, in_=psum_bc)
        gps_ctx.close()  # release gating PSUM banks

        mps = ctx.enter_context(tc.tile_pool(name="mpsum", bufs=8, space="PSUM"))

        # ------------- per-battery combine + fused matmuls ----------------
        def _vload(eng, ap, name):
            reg = eng.alloc_register(name)
            eng.reg_load(reg, ap)
            val = eng.snap(reg, donate=True)
            return nc.s_assert_within(val, 0, E - 1, skip_runtime_assert=True)

        wbs = {}

        def combine(b):
            """wb[kt] = g1*EXP[kt, e1] + g2*EXP[kt, e2], chunked per k-tile."""
            rv1 = _vload(nc.scalar, sidx[b:b + 1, 0:1], f"e1_{b}")
            rv2 = _vload(nc.vector, sidx[b:b + 1, 1:2], f"e2_{b}")
            wb = wbpool.tile([128, KT, D], BF16)
            for kt in range(KT):
                tsc = scpool.tile([128, D], BF16, tag="tsc", bufs=4)
                src1 = EXP_sb[:, kt, ds(rv1, 1), :].rearrange("p o d -> p (o d)")
                nc.scalar.activation(
                    out=tsc, in_=src1,
                    func=mybir.ActivationFunctionType.Copy,
                    scale=bcastG[:, 2 * b:2 * b + 1],
                )
                src2 = EXP_sb[:, kt, ds(rv2, 1), :].rearrange("p o d -> p (o d)")
                nc.vector.scalar_tensor_tensor(
                    out=wb[:, kt, :], in0=src2,
                    scalar=bcastGbf[:, 2 * b + 1:2 * b + 2],
                    in1=tsc, op0=AluOpType.mult, op1=AluOpType.add,
                )
            wbs[b] = wb

        def emit_battery(b):
            wb = wbs[b]
            psums = [mps.tile([128, D], F32, tag=f"pm{b % 2}_{m}", bufs=1,
                              name=f"pm_{b}_{m}")
                     for m in range(MT)]
            for kt in range(KT):
                for m in range(MT):
                    nc.tensor.matmul(
                        out=psums[m], lhsT=xt_sb[:, b, kt, ts(m, 128)],
                        rhs=wb[:, kt, :],
                        start=(kt == 0), stop=(kt == KT - 1),
                    )
            for m in range(MT):
                osb = opool.tile([128, D], BF16)
                nc.vector.tensor_copy(out=osb, in_=psums[m])
                nc.sync.dma_start(out=out_ap[b, m], in_=osb)

        combine(0)
        for b in range(BPC):
            if b + 1 < BPC:
                combine(b + 1)
            emit_battery(b)


def make_nc():
    from concourse import bacc
    nc = bacc.Bacc("TRN2", target_bir_lowering=False, debug=False,
                   num_devices=NCORES, num_swdge_queues=4)
    build_program(nc)
    if not USE_CC:
        # Register the prelude AllGather (keeps NEFF collective init +
        # preamble barrier); nobody waits on its completion.  Attach each
        # ladder round's remote-arrival wait (2 increments per arrival) to
        # its local add; the Tile scheduler's single-core sim must not see
        # these waits (they cannot be satisfied locally).
        nc._bir_kernel_barrier_sem_replica_groups.append(set(range(NCORES)))
        if os.environ.get("MOE_BARRIER"):
            # gate the first send on the prelude AllGather completion
            _POSTHOC["trig0"].wait_op(
                nc._bir_kernel_barrier_sem, nc.bir_kernel_barrier_sem_inc,
                "sem-ge", check=False)
        for add, sem, val in _POSTHOC["sum_waits"]:
            add.wait_op(sem, val, "sem-ge", check=False)
    nc.finalize()
    return nc


def prep_inputs(cycle_curve_data, cycle_numbers, DKP_embeddings,
                gate_W1, gate_b1, gate_W2, gate_b2,
                expert_W, expert_b, gen_W, gen_b):
    """Host-side layout prep. Returns per-core in_maps list."""
    f32 = np.float32
    bf16 = ml_dtypes.bfloat16
    x = np.asarray(cycle_curve_data, dtype=f32).reshape(B, L, CF)

    # xt[b, p, kt, l] = xpad[b, kt*128+p, l]
    xpad = np.zeros((B, KP, L), dtype=bf16)
    xpad[:, :CF, :] = x.transpose(0, 2, 1).astype(bf16)
    xpad[:, CF, :] = np.asarray(1.0, dtype=bf16)
    xt_all = np.ascontiguousarray(
        xpad.reshape(B, KT, 128, L).transpose(0, 2, 1, 3))

    # fold general path into every expert (gates sum to 1)
    ew_f = np.asarray(expert_W, dtype=f32) + np.asarray(gen_W, dtype=f32)[None]
    eb_f = np.asarray(expert_b, dtype=f32) + np.asarray(gen_b, dtype=f32)[None]
    ew_p = np.zeros((E, KP, D), dtype=bf16)
    ew_p[:, :CF, :] = ew_f.astype(bf16)
    ew_p[:, CF, :] = eb_f.astype(bf16)
    # ew_dram[p, kt, e, d] = ew_p[e, kt*128+p, d]
    ew_dram = np.ascontiguousarray(
        ew_p.reshape(E, KT, 128, D).transpose(2, 1, 0, 3))

    gint = np.zeros((GK, B), dtype=f32)
    gint[:DLLM, :] = np.asarray(DKP_embeddings, dtype=f32).T
    gint[DLLM, :] = np.asarray(cycle_numbers, dtype=f32)[:, 0]
    gint[DLLM + 1, :] = 1.0
    gint_dram = np.ascontiguousarray(
        gint.reshape(GKT, 128, B).transpose(1, 0, 2))

    w1p = np.zeros((GK, DFF), dtype=f32)
    w1p[:DLLM + 1, :] = np.asarray(gate_W1, dtype=f32)
    w1p[DLLM + 1, :] = np.asarray(gate_b1, dtype=f32)

    w2 = np.asarray(gate_W2, dtype=f32)
    b2v = np.asarray(gate_b2, dtype=f32).reshape(1, E)

    in_maps = []
    for c in range(NCORES):
        w1slice = w1p[:, c * DFFC:(c + 1) * DFFC]
        w1_dram = np.ascontiguousarray(
            w1slice.reshape(GKT, 128, DFFC).transpose(1, 0, 2))
        w2slice = w2[c * DFFC:(c + 1) * DFFC, :]
        w2_dram = np.ascontiguousarray(
            w2slice.reshape(DFFC // 128, 128, E).transpose(1, 0, 2))
        sel = np.zeros((B, BPC), dtype=f32)
        for i in range(BPC):
            sel[c * BPC + i, i] = 1.0
        in_maps.append({
            "xt": np.ascontiguousarray(xt_all[c * BPC:(c + 1) * BPC]),
            "ew": ew_dram,
            "gint": gint_dram,
            "w1c": w1_dram,
            "w2c": w2_dram,
            "selt": sel,
            "b2": b2v,
            "identd": np.eye(128, dtype=f32),
            "onesd": np.ones((1, 128), dtype=f32),
        })
    return in_maps


_CACHED = {}


def run(inputs, trace=False, tmpdir=None):
    """Run on the 8 NeuronCores; returns (full_output, BassKernelResults)."""
    from concourse import bass_utils
    in_maps = prep_inputs(**inputs)
    nc = _CACHED.get("nc")
    if nc is None:
        nc = make_nc()
        _CACHED["nc"] = nc
    res = bass_utils.run_bass_kernel_spmd(
        nc, in_maps, core_ids=list(range(NCORES)), trace=trace, tmpdir=tmpdir
    )
    outs = [np.asarray(r["out"]).reshape(BPC, L, D) for r in res.results]
    full = np.concatenate(outs, axis=0).astype(np.float32)
    return full, res


def kernel(**inputs):
    full, _ = run(inputs, trace=False)
    return full
